# revision 55
# baseline (speedup 1.0000x reference)
"""CARAFE (content-aware upsampling) Trainium2 kernel.

Full inputs -> shard over 8 NeuronCores (batch x image-half) -> bass/Tile
kernel per core -> gather full output.

Reference semantics:
  comp = conv1x1(x, w_comp) + b_comp                    [n,64,64,64]
  mask = conv3x3(comp, w_enc, pad=1) + b_enc            [n,100,64,64]
  m    = softmax over 25 of pixel_shuffle(mask, 2)      [n,25,128,128]
  out[n,c,i,j] = sum_k m[n,k,i,j] * xpad[n,c,i//2+p, j//2+q],  k=5p+q

v3 (73.7us -> 51.7us):
  - p-pair CARAFE matmuls: K=128 over (s=p parity, xcol) so each block
    needs 3 matmuls (2x K=128 + 1x K=64) instead of 5x K=64 (PE cost
    model charges N cols only, so K-packing is a pure win).
  - band DRAM buffers are host-zeroed ExternalInputs (no zero-fill DMAs).
  - tap-paired 3x3 conv (K=128 over (ch@dx, ch@dx+1)) via a col-shifted
    second copy of comp in partitions 64-127 (Pool copies hi from lo).
  - xTe/xTo built by XBAR DMA transposes (batched per-128-block, out AP
    3D [p][k][c]) instead of 72 PE transposes + PSUM evacs.
  - emission interleaves conv1x1/conv3x3/maskT so the DVE permute chain
    (the pre-CARAFE critical path) starts early; queue assignment tuned
    empirically (th0 scatters SP+Act, th1 SP; readbacks alternate
    Pool/Act; out DMAs SP with the last two groups' ct1 on Act).
"""
import numpy as np
import sys
from contextlib import ExitStack

sys.path.insert(0, "/opt/trn_rl_repo")

# ---------------- problem constants (hardcoded per spec) ----------------
N_B, C, H, W = 4, 256, 64, 64
CC = 64            # compressed channels
K5 = 5             # carafe kernel
S = 2              # scale
CM = K5 * K5 * S * S   # 100 mask channels
NCORES = 8
RH = H // 2        # 32 low-res rows per core
SLAB = RH + 4      # 36 x-rows per core (h0-2 .. h0+33)
NBLK = RH // 2     # 16 h-pair blocks
PIXC = RH * W      # 2048 low-res pixels per core
HO, WO = 2 * RH, 2 * W   # 64 x 128 output shard
NXT = SLAB // 2    # 18 non-overlapping xT row-pair tiles

_MM_DT = "bfloat16"

# band2 geometry: [128 (s, xcol), NBLK*2 * BTR] with BTR cols per (t, r)
BTR = W * 12           # 768: 64 w'-slots x (3 pp x 4 ab)
NHALF = 2              # two DRAM half-tensors for pipelining
TRH = NBLK // NHALF * 2    # 16 (t, r) blocks per half
B2R = TRH * BTR        # 12288: flat row stride of each half tensor


def _build_program():
    import concourse.bass as bass
    import concourse.tile as tile
    from concourse import bacc, mybir
    from concourse.ap import AP
    AF = mybir.ActivationFunctionType

    def pstep(t):
        return t[:].ap[0][0]

    f32 = mybir.dt.float32
    mmdt = getattr(mybir.dt, _MM_DT)

    nc = bacc.Bacc("TRN2", target_bir_lowering=False, debug=False,
                   num_devices=NCORES)

    # ---------------- DRAM parameters ----------------
    xs = nc.dram_tensor("xs", [C, SLAB, W], mmdt, kind="ExternalInput")
    id128 = nc.dram_tensor("id128", [128, 128], mmdt, kind="ExternalInput")
    wcT = nc.dram_tensor("wcT", [C, CC], mmdt, kind="ExternalInput")
    bc = nc.dram_tensor("bc", [CC, 1], f32, kind="ExternalInput")
    # tap-paired encoder weights [128, 6*CM]: per dy: (dx0;dx1) pair + (dx2;0)
    weT2 = nc.dram_tensor("weT2", [128, 6 * CM], mmdt, kind="ExternalInput")
    be = nc.dram_tensor("be", [CM, 1], f32, kind="ExternalInput")
    identT = nc.dram_tensor("identT", [CM, CM + 4], mmdt, kind="ExternalInput")
    # host-zeroed band halves (scatter writes diagonals; gaps must be 0)
    bndz = [nc.dram_tensor(f"bndz{th}", [128, B2R], mmdt, kind="ExternalInput")
            for th in range(NHALF)]
    out = nc.dram_tensor("out", [C, HO, WO], mmdt, kind="ExternalOutput")

    COMP_W = W + 2      # 66: comp cols with 1 zero col each side

    with tile.TileContext(nc) as tc:
        with ExitStack() as ctx:
            cpool = ctx.enter_context(tc.tile_pool(name="const", bufs=1))
            xpool = ctx.enter_context(tc.tile_pool(name="xdata", bufs=1))
            work = ctx.enter_context(tc.tile_pool(name="work", bufs=3))
            opool = ctx.enter_context(tc.tile_pool(name="oevac", bufs=2))
            pers = ctx.enter_context(tc.tile_pool(name="pers", bufs=1))
            ps_mask_p = ctx.enter_context(tc.tile_pool(name="psmask", bufs=2, space="PSUM"))
            ps_t_p = ctx.enter_context(tc.tile_pool(name="pst", bufs=2, space="PSUM"))
            ps_o_p = ctx.enter_context(tc.tile_pool(name="pso", bufs=4, space="PSUM"))

            # ---------------- input loads (x first; split SP/Act queues) ----
            # x_sb: 2 tiles [128, SLAB*W] channel-major
            t_x = [xpool.tile([128, SLAB * W], mmdt, tag=f"x{k}", name=f"x{k}") for k in range(2)]
            nc.sync.dma_start(t_x[0][:], xs.ap()[0:128, :, :])
            nc.scalar.dma_start(t_x[1][:], xs.ap()[128:256, :, :])
            t_wc = [cpool.tile([128, CC], mmdt, tag=f"wc{k}", name=f"wc{k}") for k in range(2)]
            nc.sync.dma_start(t_wc[0][:], wcT.ap()[0:128, :])
            nc.scalar.dma_start(t_wc[1][:], wcT.ap()[128:256, :])
            t_bc = cpool.tile([CC, 1], f32, tag="bc", name="bc")
            nc.sync.dma_start(t_bc[:], bc.ap())
            t_we = cpool.tile([128, 6 * CM], mmdt, tag="we", name="we")
            nc.scalar.dma_start(t_we[:], weT2.ap())
            t_be = cpool.tile([CM, 1], f32, tag="be", name="be")
            nc.sync.dma_start(t_be[:], be.ap())
            # [I_100 | sel01]: transpose-matmul rhs; cols 100..103 sum k%4
            t_id2 = cpool.tile([CM, CM + 4], mmdt, tag="id2", name="id2")

            # xT in two alignments so any row is available at base partition
            # 0 AND 64: xTe col-block k = rows (2k, 2k+1), k=0..17;
            # xTo col-block k = rows (2k+1, 2k+2), k=0..17 (block 17 = row 35
            # only, partitions 0..63).
            t_id128 = cpool.tile([128, 128], mmdt, tag="id128", name="id128")
            nc.sync.dma_start(t_id128[:], id128.ap())
            t_xTe = xpool.tile([128, NXT * C], mmdt, tag="xTe", name="xTe")
            t_xTo = xpool.tile([128, NXT * C], mmdt, tag="xTo", name="xTo")

            # ---------------- comp = 1x1 conv + bias (rows 1..34 of slab) ----
            # comp2 [128, 34*66]: partitions 0-63 = comp with zero cols 0/65;
            # partitions 64-127 = comp shifted left by 1 col (for tap pairs):
            # comp2[64+ch, r, c] = comp[ch, r, c+1].
            t_comp = pers.tile([128, (RH + 2) * COMP_W], mmdt, tag="comp", name="comp")
            compv = t_comp[:].rearrange("p (r w) -> p r w", w=COMP_W)
            nc.gpsimd.memset(compv[0:CC, :, 0:1], 0.0)
            nc.gpsimd.memset(compv[0:CC, :, COMP_W - 1:COMP_W], 0.0)
            nc.gpsimd.memset(compv[CC:128, :, COMP_W - 2:COMP_W], 0.0)

            NPIX_C = (RH + 2) * W  # 2176 pixels (rows 1..34 of slab)
            ctile = 512
            nct = (NPIX_C + ctile - 1) // ctile
            t_em = pers.tile([CM, PIXC], mmdt, tag="emask", name="emask")
            emv = t_em[:].rearrange("p (r w) -> p r w", w=W)
            mtile = 512

            def conv1x1_tile(nt):
                p0 = nt * ctile
                n = min(ctile, NPIX_C - p0)
                # shares the psmask ring (ring deps order comp vs mask conv)
                psf = ps_mask_p.tile([CM, ctile], f32, tag="ps_mask", name="ps_mask")
                ps = psf[0:CC, :]
                for k in range(2):
                    rhs = AP(t_x[k][:].tensor, W + p0, [[pstep(t_x[k]), 128], [1, n]])
                    nc.tensor.matmul(ps[:, :n], t_wc[k][:], rhs,
                                     start=(k == 0), stop=(k == 1))
                r0 = p0 // W
                nr = n // W
                psr = ps[:, :n].rearrange("p (r w) -> p r w", w=W)
                nc.scalar.activation(compv[0:CC, r0:r0 + nr, 1:1 + W], psr,
                                     func=AF.Identity, bias=t_bc[:])
                nc.gpsimd.tensor_copy(compv[CC:128, r0:r0 + nr, 0:W],
                                      compv[0:CC, r0:r0 + nr, 1:1 + W])

            def conv3x3_tile(nt):
                # mask conv 3x3 -> exp (tap-paired); emask = exp(mask)
                mr0 = nt * mtile // W   # 8 mask rows per tile
                ps = ps_mask_p.tile([CM, mtile], f32, tag="ps_mask", name="ps_mask")
                first = True
                for dy in range(3):
                    # pair (dx=0, dx=1): K=128 (lo reads comp col base+0,
                    # hi partitions read shifted copy = comp col base+1)
                    rhs = compv[:, mr0 + dy:mr0 + dy + 8, 0:W]
                    nc.tensor.matmul(ps[:], t_we[:, (2 * dy) * CM:(2 * dy + 1) * CM],
                                     rhs, start=first, stop=False)
                    first = False
                    # single dx=2: K=64
                    rhs1 = compv[0:CC, mr0 + dy:mr0 + dy + 8, 2:2 + W]
                    nc.tensor.matmul(ps[:], t_we[0:CC, (2 * dy + 1) * CM:(2 * dy + 2) * CM],
                                     rhs1, start=False, stop=(dy == 2))
                # exp(mask + be) -> emask rows 0..99
                dst = emv[0:CM, mr0:mr0 + 8, :]
                nc.scalar.activation(dst, ps[:].rearrange("p (r w) -> p r w", w=W),
                                     func=AF.Exp, bias=t_be[:])



            # ---------------- mask pipeline: all blocks -> rpp2 ------------
            # rpp2 [128 (s, w'), NBLK*2*5*12]: col = ((2t+r)*5 + qd)*12
            #   + pp*4 + 2a + b, holding normalized mask for tap p = 2pp+s,
            #   q = 4-qd at pixel (2t+r, w').  (s=1, pp=2) slots unused.
            t_rpa = pers.tile([128, NBLK * 120], mmdt, tag="rpp2", name="rpp2")
            rps = pstep(t_rpa)
            # zero the (s=1, pp=2) slots once: partitions 64.., qd strided
            rpav = t_rpa[:].rearrange("p (tr qd g) -> p tr qd g", qd=K5, g=12)
            nc.gpsimd.memset(rpav[64:128, :, :, 8:12], 0.0)

            def maskT_block(t):
                # "transpose" via matmul: emask[:, blk].T @ [I|sel]
                # -> [128 pix (r, w'), 104]: cols 0..99 masks, 100..103 Z
                psT = ps_t_p.tile([128, CM + 4], f32, tag="ps_T", name="ps_T")
                src = emv[:, 2 * t:2 * t + 2, :].rearrange("p a b -> p (a b)")
                nc.tensor.matmul(psT[:], src, t_id2[:], start=True, stop=True)

                # recip + fused normalize/permute muls on DVE, reading the
                # transpose PSUM directly. psT ch = 20p + 16-4qd + 2a + b
                # with p = 2pp + s -> ch = 40pp + 20s + 16 - 4qd + (2a+b).
                t_rz = work.tile([128, 4], f32, tag="rz", name="rz")
                nc.vector.reciprocal(t_rz[:], psT[:, CM:CM + 4])
                tps = pstep(psT)
                rzs = pstep(t_rz)
                for r in range(2):
                    for s in range(2):
                        np_ = 3 - s  # pp count
                        in0 = AP(psT[:].tensor, (64 * r) * tps + 20 * s + 16,
                                 [[tps, 64], [-4, 5], [40, np_], [1, 4]])
                        in1 = AP(t_rz[:].tensor, (64 * r) * rzs,
                                 [[rzs, 64], [0, 5], [0, np_], [1, 4]])
                        dstp = AP(t_rpa[:].tensor,
                                  (64 * s) * rps + (2 * t + r) * 60,
                                  [[rps, 64], [12, 5], [4, np_], [1, 4]])
                        nc.vector.tensor_mul(dstp, in0, in1)

            # interleave: conv3x3 tile j needs comp evac tiles j, j+1 only;
            # maskT block t needs emask tile t//4 only.  Emitting maskT
            # early unblocks the DVE permute chain (the pre-CARAFE critical
            # path) while later conv tiles still run.
            conv1x1_tile(0)
            # id2 load sits behind the first comp evac on Act (needed only
            # by the first mask transpose, ~4us later)
            nc.scalar.dma_start(t_id2[:], identT.ap())
            conv1x1_tile(1)
            for j in range(PIXC // mtile):
                if j + 2 < nct:
                    conv1x1_tile(j + 2)
                conv3x3_tile(j)
                for tb in range(4 * j, 4 * j + 4):
                    maskT_block(tb)

            # ---------------- x transposes -> xTe / xTo --------------------
            # XBAR DMA transposes (batched per-128-block): out 3D
            # [p][k][c] <- in [128, 128k+c].  Replaces 72 PE transposes +
            # PSUM evacs, freeing PE/DVE/Act for the mask pipeline.
            xes = pstep(t_xTe)
            xos = pstep(t_xTo)
            for c in range(2):
                eng = nc.sync if c == 0 else nc.scalar
                # xTe blocks 0..17: rows (2k, 2k+1)
                dste = AP(t_xTe[:].tensor, 128 * c,
                          [[xes, 128], [C, NXT], [1, 128]])
                eng.dma_start(dste, t_x[c][:], transpose=True)
                # xTo blocks 0..16: rows (2k+1, 2k+2)
                dsto = AP(t_xTo[:].tensor, 128 * c,
                          [[xos, 128], [C, NXT - 1], [1, 128]])
                eng.dma_start(dsto, t_x[c][:, 64:64 + 128 * (NXT - 1)],
                              transpose=True)
                # xTo block 17: row 35 only (64 cols < xbar tile) ->
                # PE transpose + evac, partitions 0..63
                psx = ps_o_p.tile([128, 256], f32, tag="ps_o",
                                  name=f"ps_ox{c}")
                w0 = 128 * (NXT - 1) + 64
                nc.tensor.matmul(psx[0:64, 0:128], t_x[c][:, w0:w0 + 64],
                                 t_id128[:], start=True, stop=True)
                dst17 = t_xTo[0:64, C * (NXT - 1) + 128 * c:
                              C * (NXT - 1) + 128 * (c + 1)]
                nc.vector.tensor_copy(dst17, psx[0:64, 0:128])

            # ---------------- band scatter: DRAM diagonal placement --------
            # bndz[th] flat [(64s + xcol) 128, (tr, w'slot, pp, ab) B2R].
            # For fixed (th, qd, s): walk w' diagonally (src partition +1;
            # dst flat +B2R+12), (t, r) merged (src +60, dst +768), 12 els.
            with nc.allow_non_contiguous_dma(reason="banded mask scatter"):
                for th in range(NHALF):
                    for qd in range(K5):
                        wp0 = max(0, 2 - qd)    # first valid xcol
                        jl0 = max(0, qd - 2)    # first valid w'
                        cnt = W - abs(qd - 2)
                        for s in range(2):
                            dst = AP(bndz[th].ap().tensor,
                                     (64 * s + wp0) * B2R + jl0 * 12,
                                     [[B2R + 12, cnt],   # (xcol, w') diagonal
                                      [BTR, TRH],        # (t, r)
                                      [1, 12]])          # (pp, a, b)
                            srcb = AP(t_rpa[:].tensor,
                                      (64 * s + jl0) * rps
                                      + (NBLK // NHALF * 2) * 60 * th + qd * 12,
                                      [[rps, cnt],       # w' (partition walk)
                                       [60, TRH],        # (t, r)
                                       [1, 12]])         # (pp, a, b)
                            # th0 split SP/Act for fast first-half
                            # completion; th1 all SP.
                            eng = nc.scalar if (th == 0 and qd % 2 == 1) \
                                else nc.sync
                            eng.dma_start(dst, srcb)

            # band2 SBUF [128 (s, xcol), NBLK*2*BTR]
            t_bnd = pers.tile([128, NBLK * 2 * BTR], mmdt, tag="band2", name="band2")
            bps = pstep(t_bnd)

            # ---------------- CARAFE + batched output ----------------------
            # out[ct, (a, r?, w', b)] via 3 matmuls per (t, r, ct):
            #   pp0: K=128 lhsT = xT[t+pp] rows, rhs = band2 pp slice
            #   pp2: K=64  (s=0 only)
            GRP = 2                       # blocks per output DMA
            RBG = 1                       # blocks per band readback DMA
            for t in range(NBLK):
                g, gi = t // GRP, t % GRP
                if t % RBG == 0:
                    # readback this chunk's band: DRAM -> SBUF (dense),
                    # round-robin over Pool/DVE/Act queues so chunks issue
                    # in parallel and don't stack on one sequencer.
                    th = t // (NBLK // NHALF)
                    tl = t % (NBLK // NHALF)
                    rb_eng = (nc.gpsimd, nc.scalar)[(t // RBG) % 2]
                    rb_eng.dma_start(
                        t_bnd[:, 2 * BTR * t:2 * BTR * (t + RBG)],
                        bndz[th].ap()[:, 2 * BTR * tl:2 * BTR * (tl + RBG)])
                if gi == 0:
                    t_og = [opool.tile([128, GRP * 512], mmdt, tag=f"og{c}",
                                       name=f"og{c}g{g}") for c in range(2)]
                for ct in range(2):
                    # NOTE: two accumulation groups inside one PSUM tile
                    # crash the device; keep one PSUM tile per r.
                    pso = [ps_o_p.tile([128, 256], f32, tag="ps_o",
                                       name=f"ps_o{rr}") for rr in range(2)]
                    for r in range(2):
                        t_xt = t_xTe if r == 0 else t_xTo
                        base = (2 * t + r) * BTR
                        for pp in range(3):
                            k = t + pp
                            lo = C * k + 128 * ct
                            rhs_np = 128 if pp < 2 else 64
                            rhs = AP(t_bnd[:].tensor, base + 4 * pp,
                                     [[bps, rhs_np], [2, 2],
                                      [12, W], [1, 2]])
                            nc.tensor.matmul(
                                pso[r][:],
                                t_xt[0:rhs_np, lo:lo + 128], rhs,
                                start=(pp == 0), stop=(pp == 2))
                    for r in range(2):
                        dst = t_og[ct][:, 512 * gi + 256 * r:
                                       512 * gi + 256 * (r + 1)]
                        if ct == 0:
                            nc.vector.tensor_copy(dst, pso[r][:])
                        else:
                            nc.scalar.activation(dst, pso[r][:], func=AF.Copy)
                if gi == GRP - 1:
                    # 16 output rows (hr = 4*(t-3) .. 4*t+3), cols contiguous
                    for ct in range(2):
                        dsto = AP(out.ap().tensor,
                                  ct * 128 * HO * WO + 4 * (t - GRP + 1) * WO,
                                  [[HO * WO, 128], [1, GRP * 512]])
                        eng_o = nc.scalar if (ct == 1 and g >= NBLK // GRP - 2) \
                            else nc.sync
                        eng_o.dma_start(dsto, t_og[ct][:])

    nc.compile()
    return nc


_CACHE = {}


def _get_program():
    if "nc" not in _CACHE:
        _CACHE["nc"] = _build_program()
    return _CACHE["nc"]


def host_prep(x, w_comp, b_comp, w_enc, b_enc):
    """Build per-core input maps."""
    import ml_dtypes
    bf16 = ml_dtypes.bfloat16
    x = np.asarray(x, dtype=np.float32)
    wcT = np.ascontiguousarray(
        np.asarray(w_comp, np.float32).reshape(CC, C).T).astype(bf16)
    bcv = np.asarray(b_comp, np.float32).reshape(CC, 1)
    # weT[tap, cin, cout], taps row-major (dy, dx)
    weT = np.asarray(w_enc, np.float32).reshape(CM, CC, 9).transpose(2, 1, 0)
    # tap-paired: per dy, block (2dy) = [w[dy,0]; w[dy,1]], block (2dy+1)
    # = [w[dy,2]; zeros]
    weT2 = np.zeros((128, 6, CM), np.float32)
    for dy in range(3):
        weT2[0:CC, 2 * dy] = weT[3 * dy + 0]
        weT2[CC:128, 2 * dy] = weT[3 * dy + 1]
        weT2[0:CC, 2 * dy + 1] = weT[3 * dy + 2]
    weT2 = np.ascontiguousarray(weT2.reshape(128, 6 * CM)).astype(bf16)
    be = np.asarray(b_enc, np.float32).reshape(CM, 1)
    identT = np.zeros((CM, CM + 4), np.float32)
    identT[np.arange(CM), np.arange(CM)] = 1.0
    identT[np.arange(CM), CM + np.arange(CM) % 4] = 1.0
    identT = identT.astype(bf16)
    id128m = np.eye(128, dtype=np.float32).astype(bf16)
    zero_half = np.zeros((128, B2R), bf16)

    in_maps = []
    for core in range(NCORES):
        n, half = core // 2, core % 2
        h0 = RH * half
        slab = np.zeros((C, SLAB, W), np.float32)
        r_lo, r_hi = h0 - 2, h0 + SLAB - 2       # x rows [r_lo, r_hi)
        v_lo, v_hi = max(0, r_lo), min(H, r_hi)
        slab[:, v_lo - r_lo:v_hi - r_lo, :] = x[n, :, v_lo:v_hi, :]
        slab16 = slab.astype(bf16)
        in_maps.append({"xs": slab16, "wcT": wcT, "bc": bcv, "weT2": weT2,
                        "be": be, "identT": identT, "id128": id128m,
                        "bndz0": zero_half, "bndz1": zero_half})
    return in_maps


def host_gather(results):
    out = np.empty((N_B, C, S * H, S * W), np.float32)
    for core in range(NCORES):
        n, half = core // 2, core % 2
        out[n, :, HO * half:HO * (half + 1), :] = np.asarray(
            results[core]["out"], np.float32)
    return out


def kernel(x, w_comp, b_comp, w_enc, b_enc):
    from concourse.bass_utils import run_bass_kernel_spmd
    nc = _get_program()
    in_maps = host_prep(x, w_comp, b_comp, w_enc, b_enc)
    res = run_bass_kernel_spmd(nc, in_maps, list(range(NCORES)))
    return host_gather(res.results)
